# revision 13
# baseline (speedup 1.0000x reference)
"""Trainium2 Bass kernel for nn_Attention (dense_transformer).

Sharding: 8 cores = 2 batches x 4 heads; each core computes one (batch, head)
attention (head/tensor parallel). QKV column-sharded, out-proj row-sharded per
head; host sums the per-head partial output projections and adds the bias.

The kernel is ScalarE(exp)-bound: 4096^2 exps/core at 1 elem/cycle/lane
@1.2GHz is a ~110us floor. Everything else is arranged to hide under it:

  - scores K^T Q has contraction K=64, so pairs of key-blocks run concurrently
    on the PE via 64-row tile_position packing (array rows 0:63 serve even
    key-blocks, rows 64:127 odd ones; q and k live in both partition halves).
  - exp runs on [128, 1536] psum groups (3 key-blocks x 512 queries) to
    amortize the ~310-cycle ACTIVATE overhead; the 6 score banks are per-group
    pool tiles so ACT ping-pongs one 3-bank half while the PE fills the other.
  - q/k/v'^T projections stream through the same psum rotation as pseudo-
    groups (their "exp" is a DVE cast), placed by k-block deadlines.
  - score slots interleave each pair of query chunks (lag 7) so projection
    deadlines stretch 2x and the two live [O;denom] accumulators (psum banks
    6/7, ones-column trick for the denominator) stagger their epilogues.
  - AV groups emit lazily (never ahead of ready scores in the in-order PE
    queue); the out-projection reuses each chunk's own bank after evacuation.
  - dummy matmuls warm the HAM clock gate to 2.4GHz during the input DMA and
    the exp table set loads early.

fp16 operands throughout; final absmax rel err ~5e-4 (tolerance 2e-2).
"""

from collections import deque

import numpy as np

import concourse.bass as bass
import concourse.tile as tile
from concourse import bacc, mybir
from concourse.bass_utils import run_bass_kernel_spmd

HEADS = 4
DIM_HEAD = 64
SCALE = DIM_HEAD**-0.5
B = 2
C = 256  # input channels
N = 4096  # spatial positions (64*64)
NCH = 512  # query chunk
NQC = N // NCH  # 8 query chunks
NB = 32  # 128-wide key blocks
NSLOT = NQC * NB  # 256 (chunk, key-block) score slots
F32 = mybir.dt.float32
F32R = mybir.dt.float32r
F16 = mybir.dt.float16
BF16 = mybir.dt.bfloat16

_CACHED_NC = None


def _build_nc() -> bass.Bass:
    """Per-core program; identical on all 8 cores (SPMD), data differs."""
    nc = bacc.Bacc(None, target_bir_lowering=False, debug=False)

    x = nc.declare_dram_parameter("x", [128, 2, N], F16, isOutput=False)
    wqkq = nc.declare_dram_parameter("wqkq", [128, 2, 128], F16, isOutput=False)
    wqkk = nc.declare_dram_parameter("wqkk", [128, 2, 128], F16, isOutput=False)
    wv = nc.declare_dram_parameter("wv", [128, 2, 128], F16, isOutput=False)
    wo = nc.declare_dram_parameter("wo", [DIM_HEAD, C], F32, isOutput=False)
    u = nc.declare_dram_parameter("u", [2, 128, N], F32, isOutput=True)
    dnm = nc.declare_dram_parameter("dnm", [1, N], F32, isOutput=True)

    with (
        tile.TileContext(nc) as tc,
        tc.tile_pool(name="sgp", bufs=2, space="PSUM") as sgp,
        tc.tile_pool(name="po", bufs=2, space="PSUM") as po,
        tc.tile_pool(name="singles", bufs=1) as singles,
        tc.tile_pool(name="esb", bufs=20) as esb,
        tc.tile_pool(name="osb", bufs=2) as osb,
        tc.tile_pool(name="usb", bufs=4) as usb,
    ):
        # psum banks 0-5: per-group 3-bank score tiles, pool ping-pong.
        # Separate tiles per group keep the two halves' dependencies decoupled
        # (one shared 6-bank tile made scores WAR-wait on the other half's exp).
        # psum banks 6,7: the two live per-chunk [O;dnm] accumulators; the
        # out-projection reuses each chunk's own bank after evacuation.
        warm_ps = po.tile([128, NCH], F32, tag="po", name="warm_ps")

        x_sb = singles.tile([128, 2, N], F16)
        wqkq_sb = singles.tile([128, 2, 128], F16)
        wqkk_sb = singles.tile([128, 2, 128], F16)
        wv_sb = singles.tile([128, 2, 128], F16)
        wo_sb = singles.tile([DIM_HEAD, C], F32R)
        # per x-chunk [q;q] / [k;k] / v projections, one fused cast each.
        # qkv_sb[:, j, 0, :]=q (both halves), [:, j, 1, :]=k blocks 4j..4j+3
        # (both halves), [0:64, j, 2, :]=v rows.
        qkv_sb = singles.tile([128, NQC, 2, NCH], F16)
        # v'^T blocks [m 128, d 64 | ones] per key block
        vt = singles.tile([128, NB, DIM_HEAD + 1], F16)
        warm = singles.tile([64, 64], F16)
        warm2 = singles.tile([64, 128], F16)
        scr = singles.tile([128, 1], F16)

        # ---- prologue: warm the ACT table + PE array while DMAs run ----
        nc.vector.memset(warm[:], 0.0)
        nc.vector.memset(warm2[:], 0.0)
        nc.vector.memset(vt[:, :, DIM_HEAD], 1.0)  # ones column
        nc.scalar.activation(scr[:], vt[:, 0, DIM_HEAD : DIM_HEAD + 1],
                             mybir.ActivationFunctionType.Exp)

        # Head DMAs: the first working set (wq/wk + x cols 0:512) goes on the
        # fast SP HWDGE queue in 256-col pieces so the entry-0 projections can
        # chase the transfers; everything else rides the Pool SWDGE queue in
        # parallel.  (The ACT HWDGE queue delivered weights ~5us late — the
        # exp engine's queue is left untouched.)
        nc.sync.dma_start(wqkk_sb[:], wqkk[:])
        nc.sync.dma_start(wqkq_sb[:], wqkq[:])
        nc.sync.dma_start(x_sb[:, 0, 0:256], x[:, 0, 0:256])
        nc.sync.dma_start(x_sb[:, 1, 0:256], x[:, 1, 0:256])
        nc.sync.dma_start(x_sb[:, 0, 256:512], x[:, 0, 256:512])
        nc.sync.dma_start(x_sb[:, 1, 256:512], x[:, 1, 256:512])
        nc.gpsimd.dma_start(wv_sb[:], wv[:])
        nc.gpsimd.dma_start(x_sb[:, :, 1536:2560], x[:, :, 1536:2560])
        nc.sync.dma_start(x_sb[:, :, 512:1536], x[:, :, 512:1536])
        nc.gpsimd.dma_start(x_sb[:, :, 2560:N], x[:, :, 2560:N])
        nc.gpsimd.dma_start(wo_sb[:], wo[:].bitcast(F32R))

        # warm burst: short 128-col dummies keep the PE busy (HAM clock ramp)
        # through the head-DMA window without queueing big lumps ahead of the
        # first real projections (WAW into the proj bank precedes qproj)
        for _ in range(16):
            nc.tensor.matmul(warm_ps[0:64, 0:128], warm[:], warm2[:],
                             start=True, stop=True)

        # ---- projections run through the sg-pool rotation as pseudo-groups ----
        # Each P-group j fills one sg tile: slot0=[q;q], slot1=[k;k],
        # slot2=[v;0pad] for x columns j*512..(j+1)*512, evacuated by ONE fused
        # DVE cast, so projections get the same 2-deep bank ping-pong as score
        # groups.  v'^T blocks come from the idle DMA xbar transpose engines.
        #
        # Score slots interleave each PAIR of query chunks (lag 7): chunk 2p
        # leads chunk 2p+1 by 7 slots.  This doubles the projection deadlines
        # (k block b first needed at slot ~2b) and staggers chunk epilogues so
        # the two live [O;dnm] accumulators (po banks) never collide.
        slot_list = []
        for p in range(NQC // 2):
            c0, c1 = 2 * p, 2 * p + 1
            for i in range(NB + 7):
                if i < NB:
                    slot_list.append((c0, i))
                if i >= 7:
                    slot_list.append((c1, i - 7))
        assert len(slot_list) == NSLOT

        emitted_vt = set()
        o_pend = {}  # chunk -> [ps_o, o65]
        sg_tiles = {}  # group -> psum score tile
        side = deque()

        def emit_pk(j):
            # [k;k] projection for x columns j*512..(j+1)*512
            ps = sgp.tile([128, 3, 4, 128], F32, tag="sg", name="sgp_k")
            sl = slice(j * NCH, (j + 1) * NCH)
            nc.tensor.matmul(ps[:, 1, :, :], wqkk_sb[:, 0, :], x_sb[:, 0, sl],
                             start=True, stop=False)
            nc.tensor.matmul(ps[:, 1, :, :], wqkk_sb[:, 1, :], x_sb[:, 1, sl],
                             start=False, stop=True)
            nc.vector.tensor_copy(qkv_sb[:, j, 1, :], ps[:, 1, :, :])

        def emit_pq(j):
            # [q;q] projection for chunk j (deadline = chunk j's first slot,
            # much later than the k deadline for most j — spreads PE load)
            ps = sgp.tile([128, 3, 4, 128], F32, tag="sg", name="sgp_q")
            sl = slice(j * NCH, (j + 1) * NCH)
            nc.tensor.matmul(ps[:, 0, :, :], wqkq_sb[:, 0, :], x_sb[:, 0, sl],
                             start=True, stop=False)
            nc.tensor.matmul(ps[:, 0, :, :], wqkq_sb[:, 1, :], x_sb[:, 1, sl],
                             start=False, stop=True)
            nc.vector.tensor_copy(qkv_sb[:, j, 0, :], ps[:, 0, :, :])

        def emit_pa0_fine():
            # entry-0 q+k projections in 256-col pieces interleaved by the
            # arrival order of the head x DMA pieces (h0c0, h1c0, h0c1, h1c1)
            ps = sgp.tile([128, 3, 4, 128], F32, tag="sg", name="sgp_a0")
            for c2 in range(2):
                csl = slice(c2 * 256, (c2 + 1) * 256)
                psl = slice(2 * c2, 2 * c2 + 2)
                for h in range(2):
                    nc.tensor.matmul(ps[:, 1, psl, :], wqkk_sb[:, h, :],
                                     x_sb[:, h, csl],
                                     start=(h == 0), stop=(h == 1))
                    nc.tensor.matmul(ps[:, 0, psl, :], wqkq_sb[:, h, :],
                                     x_sb[:, h, csl],
                                     start=(h == 0), stop=(h == 1))
            nc.vector.tensor_copy(qkv_sb[:, 0, 1, :], ps[:, 1, :, :])
            nc.vector.tensor_copy(qkv_sb[:, 0, 0, :], ps[:, 0, :, :])

        def emit_pb(j):
            # v'^T blocks via x-stationary matmuls (deferred: AV consumption
            # lags exp by the e3 pool depth)
            ps = sgp.tile([128, 3, 4, 128], F32, tag="sg", name="sgp_b")
            for b in range(4):
                mb = 4 * j + b
                bsl = slice(mb * 128, (mb + 1) * 128)
                nc.tensor.matmul(ps[:, 2, b, 0:DIM_HEAD], x_sb[:, 0, bsl],
                                 wv_sb[:, 0, 0:DIM_HEAD], start=True, stop=False)
                nc.tensor.matmul(ps[:, 2, b, 0:DIM_HEAD], x_sb[:, 1, bsl],
                                 wv_sb[:, 1, 0:DIM_HEAD], start=False, stop=True)
            nc.vector.tensor_copy(vt[:, 4 * j : 4 * j + 4, 0:DIM_HEAD],
                                  ps[:, 2, :, 0:DIM_HEAD])
            emitted_vt.update(range(4 * j, 4 * j + 4))

        def emit_scores(s):
            ci, mb = slot_list[s]
            g, i = divmod(s, 3)
            if i == 0:
                sg_tiles[g] = sgp.tile([128, 3, 4, 128], F32, tag="sg", name="sg")
            ps = sg_tiles[g]
            kap = qkv_sb[:, mb // 4, 1, (mb % 4) * 128 : (mb % 4 + 1) * 128]
            qap = qkv_sb[:, ci, 0, :]
            if mb % 2 == 0:
                nc.tensor.matmul(ps[:, i, :, :], kap[0:64, :], qap[0:64, :],
                                 start=True, stop=True)
            else:
                nc.tensor.matmul(ps[:, i, :, :], kap[64:128, :], qap[64:128, :],
                                 start=True, stop=True)

        def emit_act(g):
            nsl = min(3, NSLOT - 3 * g)
            e3 = esb.tile([128, 3 * NCH], F16, tag="e3", name="e3")
            nc.scalar.activation(e3[:, 0 : nsl * NCH], sg_tiles[g][:, 0:nsl, :, :],
                                 mybir.ActivationFunctionType.Exp)
            return e3

        def emit_av(g, e3, i0=0):
            # returns resume index if a new chunk would start mid-group
            lo = 3 * g
            hi = min(lo + 3, NSLOT)
            for i, s in enumerate(range(lo, hi)):
                if i < i0:
                    continue
                ci, mb = slot_list[s]
                assert mb in emitted_vt, (ci, mb)
                if mb == 0 and ci not in o_pend:
                    if i > i0:
                        return i  # split: requeue remainder behind fresh scores
                    o_pend[ci] = [po.tile([128, NCH], F32, tag="po", name="ps_o"), None]
                ps_o = o_pend[ci][0]
                nc.tensor.matmul(
                    ps_o[0 : DIM_HEAD + 1, :],
                    vt[:, mb, :],
                    e3[:, i * NCH : (i + 1) * NCH],
                    start=(mb == 0),
                    stop=(mb == NB - 1),
                )
                if mb == NB - 1:
                    # o65 evac inline: frees the po bank for the next chunk ASAP
                    ep_stage(ci, 0)
                    side.appendleft((ep_stage, (ci, 2)))
                    side.appendleft((ep_stage, (ci, 1)))
            return None

        def ep_stage(ci, st):
            ps_o, o65 = o_pend[ci][0], o_pend[ci][1]
            n0 = ci * NCH
            if st == 0:
                o65 = osb.tile([DIM_HEAD + 1, NCH], F32R, tag="o65", name="o65")
                o_pend[ci][1] = o65
                nc.vector.tensor_copy(o65[:], ps_o[0 : DIM_HEAD + 1, :])
                nc.sync.dma_start(
                    dnm[0:1, n0 : n0 + NCH],
                    o65[DIM_HEAD : DIM_HEAD + 1, :].bitcast(F32),
                )
            else:
                hf = st - 1
                nc.tensor.matmul(ps_o[:, :], wo_sb[:, hf * 128 : (hf + 1) * 128],
                                 o65[0:DIM_HEAD, :], start=True, stop=True)
                u_t = usb.tile([128, NCH], F32, tag="u", name="u_t")
                nc.vector.tensor_copy(u_t[:], ps_o[:, :])
                nc.sync.dma_start(u[hf, :, n0 : n0 + NCH], u_t[:])
                if st == 2:
                    del o_pend[ci]

        # ---- unified group stream: projections at just-in-time deadlines ----
        # Pk_j (k blocks 4j..4j+3) must precede the first score slot touching
        # those blocks (all in pair-era 0: deadlines g1..g16).  Pq_j (q chunk
        # j) is only needed at chunk j's first slot — g2..g41 — so most q
        # projections move out of the congested head entirely.  Pb_j (v'^T)
        # only gates the lagging AV drain.  One P event per entry keeps every
        # entry's PE work under the 1.51us ACT group time.
        ngroups = (NSLOT + 2) // 3  # 86 (last group has 1 slot)
        k_need = {}  # j -> first group whose scores need k-blocks 4j..4j+3
        q_need = {}  # j -> first group whose scores need q chunk j
        for s, (ci, mb) in enumerate(slot_list):
            g = s // 3
            k_need.setdefault(mb // 4, g)
            q_need.setdefault(ci, g)
        pk_at = {0: 0, 1: 1, 2: 3, 3: 5, 4: 8, 5: 11, 6: 13, 7: 15}
        pq_at = {0: 0, 1: 2, 2: 12, 3: 14, 4: 24, 5: 26, 6: 37, 7: 39}
        pb_at = {0: 4, 1: 6, 2: 7, 3: 9, 4: 10, 5: 16, 6: 17, 7: 18}
        for j in range(NQC):
            assert pk_at[j] <= k_need[j], (j, pk_at[j], k_need[j])
            assert pq_at[j] <= q_need[j], (j, pq_at[j], q_need[j])
            # vt only feeds the AV drain, which defers through the P-era
            # (bounded by the esb pool depth, 20 groups), so Pb deadlines are
            # loose; drain_av's vt_ready check defers safely regardless.
            assert pb_at[j] <= k_need[j] + 12, (j, pb_at[j], k_need[j])
        stream = []
        for t in range(ngroups):
            for j, w in pk_at.items():
                if w == t and not (j == 0 and t == 0):
                    stream.append(("Pk", j))
            for j, w in pq_at.items():
                if w == t and not (j == 0 and t == 0):
                    stream.append(("Pq", j))
            for j, w in pb_at.items():
                if w == t:
                    stream.append(("Pb", j))
            stream.append(("R", t))
        stream.insert(0, ("P0", 0))

        av_q = deque()  # (g, e3, i0) awaiting AV emission

        def vt_ready(g):
            lo, hi = 3 * g, min(3 * g + 3, NSLOT)
            return all(slot_list[s][1] in emitted_vt for s in range(lo, hi))

        def drain_av(cap):
            n_av = 0
            while av_q and n_av < cap and vt_ready(av_q[0][0]):
                g, e3, i0 = av_q.popleft()
                res = emit_av(g, e3, i0)
                if res is not None:
                    av_q.appendleft((g, e3, res))
                    break
                n_av += 1

        had_p = False
        for ent, (kind, t) in enumerate(stream):
            if 1 <= ent <= 6:
                # keep the PE busy through early x/cast waits so the HAM
                # clock gate reaches 2.4GHz before the projection era
                for _ in range(2):
                    nc.tensor.matmul(warm_ps[0:64, 0:128], warm[:], warm2[:],
                                     start=True, stop=True)
            if kind == "P0":
                emit_pa0_fine()
                had_p = True
                continue
            if kind == "Pk":
                emit_pk(t)
                had_p = True
                continue
            if kind == "Pq":
                emit_pq(t)
                had_p = True
                continue
            if kind == "Pb":
                emit_pb(t)
                had_p = True
                continue
            for s in range(3 * t, min(3 * t + 3, NSLOT)):
                emit_scores(s)
            av_q.append((t, emit_act(t), 0))
            # AV defers through the cold-clock/projection era (exp pipeline
            # depth = esb bufs bounds the backlog), catches up at 2 groups
            # per step in the warm steady state, and drains hard near the
            # end so almost nothing remains after the last ACTIVATE.
            if t < 18:
                cap = 0 if had_p else 1
            elif t < 70:
                cap = 2
            else:
                cap = 6
            drain_av(cap)
            for _ in range(1 if had_p else 2):
                if side:
                    fn, args = side.popleft()
                    fn(*args)
            had_p = False
        while av_q:
            drain_av(99)
        while side:
            fn, args = side.popleft()
            fn(*args)

    nc.compile()
    return nc


def _get_nc() -> bass.Bass:
    global _CACHED_NC
    if _CACHED_NC is None:
        _CACHED_NC = _build_nc()
    return _CACHED_NC


def _stripe_kxm(w: np.ndarray, dtype) -> np.ndarray:
    """[256, M] -> [128, 2, M] k-subtile layout (c = t*128 + p)."""
    return np.ascontiguousarray(w.reshape(2, 128, -1).transpose(1, 0, 2)).astype(dtype)


def make_in_maps(x, w_qkv, w_out):
    x2 = np.ascontiguousarray(
        np.asarray(x).reshape(B, 2, 128, N).transpose(0, 2, 1, 3)
    ).astype(np.float16)  # [B, 128, 2, N]
    in_maps = []
    for core in range(8):
        b, h = divmod(core, HEADS)
        hs = slice(h * DIM_HEAD, (h + 1) * DIM_HEAD)
        wq_ = (w_qkv[0 * C :][hs, :] * SCALE).T  # [256, 64], scale folded
        wk_ = w_qkv[1 * C :][hs, :].T
        wv_ = np.concatenate(
            [w_qkv[2 * C :][hs, :].T, np.zeros((C, 128 - DIM_HEAD), np.float32)],
            axis=1,
        )
        wo_ = w_out[:, hs].T  # [64, 256]
        wqkq_ = np.concatenate([wq_, wq_], axis=1)  # [256, 128] dup
        wqkk_ = np.concatenate([wk_, wk_], axis=1)
        in_maps.append(
            {
                "x": x2[b],
                "wqkq": _stripe_kxm(wqkq_, np.float16),
                "wqkk": _stripe_kxm(wqkk_, np.float16),
                "wv": _stripe_kxm(wv_, np.float16),
                "wo": np.ascontiguousarray(wo_, dtype=np.float32),
            }
        )
    return in_maps


def combine(results, b_out):
    out = np.zeros((B, C, N), dtype=np.float32)
    for core in range(8):
        b, _h = divmod(core, HEADS)
        r = results[core]
        out[b] += r["u"].reshape(C, N) / r["dnm"].reshape(1, N)
    out += b_out.astype(np.float32)[None, :, None]
    return out.reshape(B, C, 64, 64)


def kernel(x, w_qkv, w_out, b_out, _run_kwargs=None):
    nc = _get_nc()
    in_maps = make_in_maps(np.asarray(x), np.asarray(w_qkv), np.asarray(w_out))
    kw = _run_kwargs or {}
    res = run_bass_kernel_spmd(nc, in_maps, list(range(8)), **kw)
    out = combine(res.results, np.asarray(b_out))
    kernel.last_result = res
    return out

